# revision 70
# baseline (speedup 1.0000x reference)
"""Trainium2 Bass kernel for IntraRegionLoss (masked softmax-CE loss, both directions).

Pure data parallel over the batch dim (8 batches/core on 8 cores). Per core the
device computes the only O(B*N^2) quantities:
  - S0[r] = sum(exp(row r))  for all 16 matrices (2 directions x 8 batches)
  - m[r]  = max(row r)       for the few self-pointing rows (label == -1),
                             fetched via an indirect row gather (64 slots/dir)
The O(B*N) epilogue (log, label gather, diagonal-overwrite correction, masked
mean) runs on host in f64.

Engine schedule per core (v1 CoreSim cost model figures; ~120us total,
ACT-bound at 97% busy):
  stream: one [128,1024] segment DMA per 128-row block; transfers serialize
    per issuing queue (~1579ns f32 / ~790ns via SWDGE f32->bf16 cast), and
    the SP and Pool queues run in parallel. Tiles 0-1 arrive bf16 via
    Pool-cast so chunked exps start at ~1.6us with no supply gap; tile 2
    whole on SP; tiles 3..13 split evens/odds; tiles 14-15 split with
    per-chunk sems.
  ACT: the bottleneck engine - exp of every element (f32/bf16 -> bf16,
    0.83ns/elem). Head/tail tiles are exp'd in 1-2-segment chunks to chase
    the DMA ramp / feed the DVE drain; the Exp table is preloaded via a
    scale=0 dummy; the final two S0 columns come from fused exp+accum so
    the output dump waits on no DVE work.
  DVE: per tile, two bf16 pairwise-adds (2x DVE mode) then a f32 reduce_sum
    -> S0 column block (~5.6us/tile), plus the self-row max. TRN2 engines do
    not interlock, so every same-engine RAW/WAR carries an explicit
    semaphore edge (~100ns prop stalls, hidden by ACT pacing).
Raw Bass blocks with per-tile-per-queue semaphores: DMA completions are NOT
assumed FIFO within a queue (observed reordering on HW), so every wait is an
order-independent count. Only ONE indirect (gather) DMA may exist in the
program - a second one crashes the exec unit - hence the concatenated
logits tensor and single 128-slot gather.
"""

from contextlib import ExitStack

import numpy as np

B, N = 64, 1024
NCORES = 8
BL = B // NCORES            # batches per core (8)
P = 128                     # partitions
TILES = 2 * BL              # 16 matrices per core: 0..7 succ, 8..15 pred
SEGS = N // P               # 8 row-blocks per matrix
SLOTS = 64                  # self-row gather slots per direction
STREAM_SLOTS = 3
EXP_SLOTS = 2

# DMA schedule: tiles 0-1 stream as bf16 via Pool-queue casting DMAs (SWDGE
# casts f32->bf16 in flight; half the output bytes = half the queue
# occupancy), so the ramp needs only one queue and ACT's chunked exps start
# at ~2.7us with no tile-1 supply gap. Tile 2 goes whole on SP while Pool
# catches up (self-row gather); tiles 3..14 split evens/odds across SP/Pool;
# tile 15 splits with per-chunk sems so DVE can chase the drain.
SP_SEGS = {0: (), 1: (), 2: tuple(range(SEGS))}
POOL_SEGS = {0: tuple(range(SEGS)), 1: tuple(range(SEGS)), 2: ()}
for _t in range(3, TILES):
    SP_SEGS[_t] = (0, 2, 4, 6)
    POOL_SEGS[_t] = (1, 3, 5, 7)

# First/last tiles are exp'd in small chunks so ACT can chase the ramp
# DMAs / DVE can chase the drain. Per-chunk semaphores keep DMA completion
# waits order-independent. Tile 0 leads with single-segment chunks so the
# first exp starts as soon as segment 0 lands.
CH = 2                      # segments per chunk (tail tiles)
NCH = SEGS // CH            # 4 chunks
CHUNKED_HEAD = (0, 1)       # pool-cast bf16 tiles, chunk-exp'd
HEAD_CHUNKS = {
    0: ((0,), (1,), (2, 3), (4, 5), (6, 7)),
    1: ((0, 1), (2, 3), (4, 5), (6, 7)),
}
CHUNKED_TAIL = (TILES - 2, TILES - 1)   # exp'd in CH-seg chunks, DVE chases

# Schraudolph fast-exp constants (sum for tile 1's segs 4..7 runs on DVE):
# exp(x) ~= bitcast_f32(int32(SCHR_A*x + SCHR_B)). SCHR_B is tuned so the
# expected bias of sum(exp) over x~N(0,1) is ~0 under round-to-nearest.
SCHR_A = float(2 ** 23 / np.log(2))
SCHR_B = 1064870797.0
# middle tiles whose segs 6-7 also go to DVE-Schraudolph (exp covers 0..5)
SHED_TILES = (6, 11)


def _build_program():
    import concourse.bass as bass
    import concourse.mybir as mybir

    f32 = mybir.dt.float32
    bf16 = mybir.dt.bfloat16
    i32 = mybir.dt.int32
    AX = mybir.AxisListType.X
    ACT = mybir.ActivationFunctionType

    nc = bass.Bass()
    # succ (matrices 0..7) and pred (8..15) concatenated: only ONE indirect
    # gather is allowed per program (a second one crashes the exec unit), so
    # both directions' self rows must come from a single DRAM tensor.
    both = nc.declare_dram_parameter("logits", [TILES, N, N], f32, isOutput=False)
    offs_d = nc.declare_dram_parameter("self_row_offs", [P, 1], i32, isOutput=False)
    s0_d = nc.declare_dram_parameter("s0_out", [P, TILES * SEGS], f32, isOutput=True)
    smax_d = nc.declare_dram_parameter("smax_out", [P, 1], f32, isOutput=True)

    def seg_src(t, j):
        return both[t, j * P:(j + 1) * P, :]

    with ExitStack() as ctx:
        stream = [
            ctx.enter_context(nc.sbuf_tensor(f"stream{i}", [P, SEGS, N], f32))
            for i in range(STREAM_SLOTS)
        ]
        stream_bf = [
            ctx.enter_context(nc.sbuf_tensor(f"streambf{i}", [P, SEGS, N], bf16))
            for i in range(len(CHUNKED_HEAD))
        ]
        expt = [
            ctx.enter_context(nc.sbuf_tensor(f"exp{i}", [P, SEGS, N], bf16))
            for i in range(EXP_SLOTS)
        ]
        s1 = ctx.enter_context(nc.sbuf_tensor("s1", [P, SEGS, N // 2], bf16))
        s2 = ctx.enter_context(nc.sbuf_tensor("s2", [P, SEGS, N // 4], bf16))
        schr = ctx.enter_context(nc.sbuf_tensor("schr", [P, SEGS // 2, N], i32))
        s0 = ctx.enter_context(nc.sbuf_tensor("s0", [P, TILES * SEGS], f32))
        dummy = ctx.enter_context(nc.sbuf_tensor("atl_dummy", [P, 1], f32))
        rows = ctx.enter_context(nc.sbuf_tensor("rows", [P, N], f32))
        smax = ctx.enter_context(nc.sbuf_tensor("smax", [P, 1], f32))
        offs_t = ctx.enter_context(nc.sbuf_tensor("offs", [P, 1], i32))

        sp_tile = [
            ctx.enter_context(nc.semaphore(f"sp_t{t}")) for t in range(TILES)
        ]
        pool_tile = [
            ctx.enter_context(nc.semaphore(f"pool_t{t}")) for t in range(TILES)
        ]
        # per-chunk DMA-completion sems for the chunk-exp'd head/tail tiles
        pool_ch = {
            t: [
                ctx.enter_context(nc.semaphore(f"pl_c{t}_{c}"))
                for c in range(len(HEAD_CHUNKS[t]))
            ]
            for t in CHUNKED_HEAD
        }
        sp_chT = {
            t: [ctx.enter_context(nc.semaphore(f"sp_T{t}_{c}")) for c in range(NCH)]
            for t in CHUNKED_TAIL
        }
        pool_chT = {
            t: [ctx.enter_context(nc.semaphore(f"pl_T{t}_{c}")) for c in range(NCH)]
            for t in CHUNKED_TAIL
        }
        act_last = ctx.enter_context(nc.semaphore("act_last"))
        schr_sem = ctx.enter_context(nc.semaphore("schr_sem"))
        schr_done = ctx.enter_context(nc.semaphore("schr_done"))
        aux_sem = ctx.enter_context(nc.semaphore("aux_sem"))
        gather_sem = ctx.enter_context(nc.semaphore("gather_sem"))
        act_done = ctx.enter_context(nc.semaphore("act_done"))
        chain_a = ctx.enter_context(nc.semaphore("chain_a"))
        chain_b = ctx.enter_context(nc.semaphore("chain_b"))
        dve_done = ctx.enter_context(nc.semaphore("dve_done"))
        smax_sem = ctx.enter_context(nc.semaphore("smax_sem"))
        out_sem = ctx.enter_context(nc.semaphore("out_sem"))
        block = ctx.enter_context(nc.Block())

        def seg_dst(t, j):
            if t in CHUNKED_HEAD:
                return stream_bf[t][:, j:j + 1, :]
            return stream[(t - 2) % STREAM_SLOTS][:, j:j + 1, :]

        def head_chunk_of(t, j):
            for c, segs in enumerate(HEAD_CHUNKS[t]):
                if j in segs:
                    return c
            raise ValueError((t, j))

        def seg_sem(t, j, is_sp):
            if t in CHUNKED_HEAD:
                return pool_ch[t][head_chunk_of(t, j)]
            if t in CHUNKED_TAIL:
                return (sp_chT if is_sp else pool_chT)[t][j // CH]
            return sp_tile[t] if is_sp else pool_tile[t]

        def slot_guard(engine, t):
            # f32 ring slot (t-2)%3 reused from tile t-3: wait for its exp,
            # and for DVE's Schraudolph pass1 if that tile shed segs 6-7.
            if t - STREAM_SLOTS >= 2:
                engine.wait_ge(act_done, t - STREAM_SLOTS + 1)
                prev = t - STREAM_SLOTS
                if prev in SHED_TILES:
                    n_p1 = 2 + 1 + SHED_TILES.index(prev)  # t1's 2 + earlier sheds
                    engine.wait_ge(schr_sem, n_p1)

        @block.sync
        def _(sync):
            # aux (self-row offsets) rides AFTER tile 2: tile 2's delivery
            # gates the ACT seam, while the gather that needs aux has ~20us
            # of margin (it runs after Pool's head casts).
            for t in range(TILES):
                if t == 3:
                    sync.dma_start(
                        out=offs_t[:], in_=offs_d[:]
                    ).then_inc(aux_sem, 16)
                if not SP_SEGS[t]:
                    continue
                slot_guard(sync, t)
                for j in SP_SEGS[t]:
                    sync.dma_start(
                        out=seg_dst(t, j), in_=seg_src(t, j)
                    ).then_inc(seg_sem(t, j, True), 16)
            sync.wait_ge(smax_sem, 1)
            sync.dma_start(out=smax_d[:], in_=smax[:]).then_inc(out_sem, 16)
            # staged S0 dumps overlapping the DVE drain: tiles 0..13, then 14,
            # then 15 (dve_done: 14 fulls, then 4+4 tail chunks)
            c14 = (TILES - 2) * SEGS
            c15 = (TILES - 1) * SEGS
            sync.wait_ge(dve_done, TILES - 2)
            sync.wait_ge(schr_done, 2 + len(SHED_TILES))
            sync.dma_start(out=s0_d[:, 0:c14], in_=s0[:, 0:c14]).then_inc(out_sem, 16)
            sync.wait_ge(dve_done, TILES - 2 + NCH)
            sync.dma_start(out=s0_d[:, c14:c15], in_=s0[:, c14:c15]).then_inc(out_sem, 16)
            cF = TILES * SEGS - CH  # first ACT-fused column
            sync.wait_ge(act_last, 2 * NCH + 1)  # ACT-fused final columns
            sync.dma_start(out=s0_d[:, cF:], in_=s0[:, cF:]).then_inc(out_sem, 16)
            sync.wait_ge(dve_done, TILES - 2 + 2 * NCH - 1)
            sync.dma_start(out=s0_d[:, c15:cF], in_=s0[:, c15:cF]).then_inc(out_sem, 16)
            sync.wait_ge(out_sem, 80)

        @block.gpsimd
        def _(gpsimd):
            # head tiles (casting) first, then the self-row gather, then the
            # odd-segment share of the remaining tiles.
            for t in CHUNKED_HEAD:
                for j in POOL_SEGS[t]:
                    gpsimd.dma_start(
                        out=seg_dst(t, j), in_=seg_src(t, j)
                    ).then_inc(seg_sem(t, j, False), 16)
            for t in range(3, TILES):
                if t == 5:
                    # self-row gather tucked after tile 4's odds: aux (on SP,
                    # after tile 2) is long since loaded, and the smax
                    # consumer on DVE runs much later.
                    gpsimd.wait_ge(aux_sem, 16)
                    gpsimd.indirect_dma_start(
                        out=rows[:, :],
                        out_offset=None,
                        in_=both[:].rearrange("a b c -> (a b) c"),
                        in_offset=bass.IndirectOffsetOnAxis(ap=offs_t[:, :], axis=0),
                    ).then_inc(gather_sem, 16)
                if not POOL_SEGS[t]:
                    continue
                slot_guard(gpsimd, t)
                for j in POOL_SEGS[t]:
                    gpsimd.dma_start(
                        out=seg_dst(t, j), in_=seg_src(t, j)
                    ).then_inc(seg_sem(t, j, False), 16)

        @block.scalar
        def _(scalar):
            # preload the Exp activation table during the DMA ramp (Bacc
            # places the InstLoadActFuncSet before this dummy); scale=0 makes
            # the result exp(0)=1 regardless of SBUF garbage.
            nc.scalar.activation(dummy[:, 0:1], dummy[:, 0:1], ACT.Exp, scale=0.0)
            # head tiles: chunked exp (bf16 in) chasing the casting DMAs.
            # Tile 1's second half (segs 4..7) is NOT exp'd here - DVE
            # computes those row sums via the Schraudolph bit-trick exp.
            for t in CHUNKED_HEAD:
                chunks = HEAD_CHUNKS[t][:2] if t == 1 else HEAD_CHUNKS[t]
                for c, segs in enumerate(chunks):
                    j0, j1 = segs[0], segs[-1] + 1
                    inst = nc.scalar.activation(
                        expt[t % EXP_SLOTS][:, j0:j1, :],
                        stream_bf[t][:, j0:j1, :],
                        ACT.Exp,
                    )._wait_ge(pool_ch[t][c], 16 * len(segs))
                    if c == len(chunks) - 1:
                        inst.then_inc(act_done, 1)
            for t in range(2, TILES - 2):
                if t >= EXP_SLOTS:
                    scalar.wait_ge(chain_a, t - EXP_SLOTS + 1)
                if SP_SEGS[t]:
                    scalar.wait_ge(sp_tile[t], 16 * len(SP_SEGS[t]))
                if POOL_SEGS[t]:
                    scalar.wait_ge(pool_tile[t], 16 * len(POOL_SEGS[t]))
                je = 6 if t in SHED_TILES else SEGS  # shed tiles: segs 6-7 on DVE
                nc.scalar.activation(
                    expt[t % EXP_SLOTS][:, 0:je, :],
                    stream[(t - 2) % STREAM_SLOTS][:, 0:je, :],
                    ACT.Exp,
                ).then_inc(act_done, 1)
            # tail tiles: chunked exp so DVE can chase the drain; the very
            # last chunk is fused exp+accum on ACT (one accum column per
            # segment) so the final S0 columns don't wait on DVE at all.
            for t in CHUNKED_TAIL:
                scalar.wait_ge(chain_a, t - 1)  # exp slot free (A(t-2) done)
                last_tile = t == TILES - 1
                for c in range(NCH):
                    if last_tile and c == NCH - 1:
                        for j in (c * CH, c * CH + 1):
                            sem = sp_chT[t][c] if j % 2 == 0 else pool_chT[t][c]
                            nc.scalar.activation(
                                expt[t % EXP_SLOTS][:, j:j + 1, :],
                                stream[(t - 2) % STREAM_SLOTS][:, j:j + 1, :],
                                ACT.Exp,
                                accum_out=s0[:, t * SEGS + j:t * SEGS + j + 1],
                            )._wait_ge(sem, 16).then_inc(act_last, 1)
                        continue
                    scalar.wait_ge(sp_chT[t][c], 16)
                    nc.scalar.activation(
                        expt[t % EXP_SLOTS][:, c * CH:(c + 1) * CH, :],
                        stream[(t - 2) % STREAM_SLOTS][:, c * CH:(c + 1) * CH, :],
                        ACT.Exp,
                    )._wait_ge(pool_chT[t][c], 16).then_inc(act_last, 1)

        @block.vector
        def _(vector):
            # A: halves-add exp tile -> s1; B: halves-add s1 -> s2;
            # C: reduce s2 -> S0 columns. Chain semaphores give every
            # same-engine RAW/WAR an explicit edge (~100ns prop stall each).
            # The self-row max slots into tile 3's slack (the gather lands
            # at ~15us, after the head casts).
            n_full = TILES - len(CHUNKED_TAIL)  # full-tile A/B/C count (14)
            H = SEGS // 2
            for t in range(n_full):
                if t == 7:
                    nc.vector.reduce_max(
                        smax[:, 0:1], rows[:, :], axis=AX
                    )._wait_ge(gather_sem, 16).then_inc(smax_sem, 1)
                e = expt[t % EXP_SLOTS]
                # tiles with Schraudolph segs have fewer exp'd segs to sum
                nco = H if t == 1 else (6 if t in SHED_TILES else SEGS)
                hs = slice(0, nco)
                if t >= 1:
                    vector.wait_ge(chain_b, t)  # s1 free (B(t-1) read it)
                nc.vector.tensor_add(
                    s1[:, hs, :], e[:, hs, 0:N // 2], e[:, hs, N // 2:N]
                )._wait_ge(act_done, t + 1).then_inc(chain_a, 1)
                if t >= 1:
                    vector.wait_ge(dve_done, t)  # s2 free (C(t-1) read it)
                nc.vector.tensor_add(
                    s2[:, hs, :], s1[:, hs, 0:N // 4], s1[:, hs, N // 4:N // 2]
                )._wait_ge(chain_a, t + 1).then_inc(chain_b, 1)
                nc.vector.reduce_sum(
                    s0[:, t * SEGS:t * SEGS + nco], s2[:, hs, :], axis=AX
                )._wait_ge(chain_b, t + 1).then_inc(dve_done, 1)
                # Schraudolph exp-sum for tile 1's second half, nibbled into
                # the per-tile slack: exp(x) ~= bitcast_f32(i32(SCHR_A*x +
                # SCHR_B)); pass1 is a fused mul-add with convert-on-write,
                # pass2 reduces the bits reinterpreted as f32.
                if t in (2, 3):
                    p = t - 2
                    nc.vector.tensor_scalar(
                        schr[:, 2 * p:2 * p + 2, :],
                        stream_bf[1][:, H + 2 * p:H + 2 * p + 2, :],
                        SCHR_A, SCHR_B,
                        mybir.AluOpType.mult, mybir.AluOpType.add,
                    )._wait_ge(pool_ch[1][2 + p], 32).then_inc(schr_sem, 1)
                if t in (4, 5):
                    p = t - 4
                    nc.vector.reduce_sum(
                        s0[:, SEGS + H + 2 * p:SEGS + H + 2 * p + 2],
                        schr[:, 2 * p:2 * p + 2, :].bitcast(f32), axis=AX,
                    )._wait_ge(schr_sem, p + 1).then_inc(schr_done, 1)
                # shed tiles: pass1 one iteration EARLY (the segs are DMA'd
                # long before exp(t) finishes - wait the seg sems directly)
                # so the ring-slot guard releases promptly; pass2 next
                # iteration. schr halves recycle: tile1's pass2 p0/p1 freed
                # them at iterations 4/5.
                if t + 1 in SHED_TILES:
                    i_sh = SHED_TILES.index(t + 1)
                    half = slice(2 * (i_sh % 2), 2 * (i_sh % 2) + 2)
                    vector.wait_ge(pool_tile[t + 1], 16 * len(POOL_SEGS[t + 1]))
                    # half (i_sh%2) was last read by the (i_sh)-th schr_done
                    vector.wait_ge(schr_done, i_sh + 1)
                    nc.vector.tensor_scalar(
                        schr[:, half, :],
                        stream[(t + 1 - 2) % STREAM_SLOTS][:, 6:8, :],
                        SCHR_A, SCHR_B,
                        mybir.AluOpType.mult, mybir.AluOpType.add,
                    )._wait_ge(sp_tile[t + 1], 16 * len(SP_SEGS[t + 1])).then_inc(
                        schr_sem, 1
                    )
                if t in SHED_TILES:
                    i_sh = SHED_TILES.index(t)
                    half = slice(2 * (i_sh % 2), 2 * (i_sh % 2) + 2)
                    nc.vector.reduce_sum(
                        s0[:, t * SEGS + 6:t * SEGS + 8],
                        schr[:, half, :].bitcast(f32), axis=AX,
                    )._wait_ge(schr_sem, 3 + i_sh).then_inc(schr_done, 1)
            # tail tiles: per-chunk A/B/C on disjoint s1/s2 slices, chasing
            # the chunked exps (act_last counts chunks across both tiles).
            vector.wait_ge(chain_b, n_full)   # s1 free (B(13) done)
            vector.wait_ge(dve_done, n_full)  # s2 free (C(13) done)
            for i, t in enumerate(CHUNKED_TAIL):
                e = expt[t % EXP_SLOTS]
                for c in range(NCH):
                    if t == TILES - 1 and c == NCH - 1:
                        continue  # last chunk is ACT-fused
                    k = i * NCH + c            # global tail-chunk index
                    sl_ = slice(c * CH, (c + 1) * CH)
                    if i >= 1:
                        # s1/s2 slice WAR vs the previous tail tile's chunks
                        vector.wait_ge(chain_b, n_full + (i - 1) * NCH + c + 1)
                        vector.wait_ge(dve_done, n_full + (i - 1) * NCH + c + 1)
                    nc.vector.tensor_add(
                        s1[:, sl_, :], e[:, sl_, 0:N // 2], e[:, sl_, N // 2:N]
                    )._wait_ge(act_last, k + 1).then_inc(chain_a, 1)
                    nc.vector.tensor_add(
                        s2[:, sl_, :], s1[:, sl_, 0:N // 4], s1[:, sl_, N // 4:N // 2]
                    )._wait_ge(chain_a, n_full + k + 1).then_inc(chain_b, 1)
                    nc.vector.reduce_sum(
                        s0[:, t * SEGS + c * CH:t * SEGS + (c + 1) * CH],
                        s2[:, sl_, :], axis=AX,
                    )._wait_ge(chain_b, n_full + k + 1).then_inc(dve_done, 1)

    return nc


def _host_self_rows(labels, line_mask):
    """Self-pointing (label == -1) row bookkeeping for one direction.

    labels: [BL, N] int64 (one core's shard). Returns (offsets[SLOTS,1] i32 of
    flattened row indices b*N+row, slot list [(b, row)], overflow list).
    """
    bs, rs = np.nonzero(labels == -1)
    offs = np.zeros((SLOTS, 1), dtype=np.int32)
    slot_map = []
    n = min(len(bs), SLOTS)
    for i in range(n):
        offs[i, 0] = bs[i] * N + rs[i]
        slot_map.append((int(bs[i]), int(rs[i])))
    overflow = [(int(b), int(r)) for b, r in zip(bs[n:], rs[n:])]
    return offs, slot_map, overflow


def kernel(successor_logits, successor_labels, predecessor_logits,
           predecessor_labels, line_mask, pred_weight):
    from concourse.bass_utils import run_bass_kernel_spmd

    sl = np.ascontiguousarray(np.asarray(successor_logits, dtype=np.float32))
    pl = np.ascontiguousarray(np.asarray(predecessor_logits, dtype=np.float32))
    s_lbl = np.asarray(successor_labels).astype(np.int64)
    p_lbl = np.asarray(predecessor_labels).astype(np.int64)
    lm = np.asarray(line_mask).astype(bool)
    pw = float(np.asarray(pred_weight))

    nc = _build_program()

    in_maps = []
    meta = []
    for core in range(NCORES):
        sli = slice(core * BL, (core + 1) * BL)
        off_s, map_s, ovf_s = _host_self_rows(s_lbl[sli], lm[sli])
        off_p, map_p, ovf_p = _host_self_rows(p_lbl[sli], lm[sli])
        off_p = off_p + np.int32(BL * N)  # pred matrices sit at rows BL*N..
        off_p[len(map_p):] = 0            # keep padding slots at row 0
        in_maps.append({
            "logits": np.concatenate([sl[sli], pl[sli]], axis=0),
            "self_row_offs": np.concatenate([off_s, off_p], axis=0),
        })
        meta.append((map_s, ovf_s, map_p, ovf_p))

    res = run_bass_kernel_spmd(nc, in_maps, list(range(NCORES)))

    # --- host epilogue (O(B*N), f64) ---
    # Device S0 layout: s0[p, t*SEGS + j] = sum(exp(row 128*j+p)) of matrix t
    # (t<8: succ batch t, t>=8: pred batch t-8) of this core's shard.
    s0_succ = np.empty((B, N), dtype=np.float64)
    s0_pred = np.empty((B, N), dtype=np.float64)
    m_succ = {}
    m_pred = {}
    for core in range(NCORES):
        s0_c = np.asarray(res.results[core]["s0_out"], dtype=np.float64)
        arr = s0_c.reshape(P, TILES, SEGS).transpose(1, 2, 0)  # [t, j, p]
        arr = arr.reshape(TILES, N)                            # row = 128j+p
        s0_succ[core * BL:(core + 1) * BL] = arr[:BL]
        s0_pred[core * BL:(core + 1) * BL] = arr[BL:]

        smax_c = np.asarray(res.results[core]["smax_out"], dtype=np.float64)
        map_s, ovf_s, map_p, ovf_p = meta[core]
        for i, (b, r) in enumerate(map_s):
            m_succ[(core * BL + b, r)] = smax_c[i, 0]
        for i, (b, r) in enumerate(map_p):
            m_pred[(core * BL + b, r)] = smax_c[SLOTS + i, 0]
        # overflow fallback (astronomically rare): host-computed row max
        for b, r in ovf_s:
            m_succ[(core * BL + b, r)] = float(sl[core * BL + b, r].max())
        for b, r in ovf_p:
            m_pred[(core * BL + b, r)] = float(pl[core * BL + b, r].max())

    def direction_loss(logits, labels, s0, m_map):
        is_self = labels == -1
        idx = np.arange(N)[None, :]
        lbl_fixed = np.where(is_self, idx, labels)
        lbl_fixed = np.clip(lbl_fixed, 0, N - 1)
        g = np.take_along_axis(
            logits, lbl_fixed[:, :, None].astype(np.int64), axis=2
        )[..., 0].astype(np.float64)
        nll = np.log(s0) - g
        # diagonal-overwrite correction for self-pointing valid rows
        bs, rs = np.nonzero(is_self & lm)
        for b, r in zip(bs, rs):
            m = m_map[(int(b), int(r))]
            s_eff = s0[b, r] - np.exp(g[b, r]) + np.exp(m + 1.0)
            nll[b, r] = np.log(s_eff) - (m + 1.0)
        valid = lm.astype(np.float64)
        denom = max(valid.sum(), 1.0)
        return float((nll * valid).sum() / denom)

    succ_loss = direction_loss(sl, s_lbl, s0_succ, m_succ)
    pred_loss = direction_loss(pl, p_lbl, s0_pred, m_pred)
    num_valid = int(lm.sum())
    total_loss = succ_loss + pw * pred_loss
    return (
        np.float32(total_loss),
        np.float32(succ_loss),
        np.float32(pred_loss),
        np.int32(num_valid),
    )
